# Initial kernel scaffold
#
"""MoE FFN (BertGeneration-style) on 8 TRN2 NeuronCores, expert-parallel.

Problem: 8192 tokens, expert = task_id % 8, per-expert FFN
(768 -> 3072 gelu -> 768) + residual + per-expert LayerNorm.

Strategy: routing (dispatch/combine) is a host-side permutation; each of the
8 cores runs one expert's FFN over its 1024-token block.  Matmuls run in
fp8 (e4m3) with perf_mode=DoubleRow: the PE packs two 128-deep k-slices per
pass (256-deep contraction), roughly halving tensor-engine time vs fp32r.
The residual path and LayerNorm stay fp32 (x is added unquantized), so the
fp8 quantization error only enters through y = FFN(x), whose magnitude is
~0.2 of the residual -- overall rel err ~1.4e-2 vs the 2e-2 gate.

On-chip per core:
  phase 1:  hT[i, m] = gelu(sum_k W1[k, i] * xT[k, m] + b1[i])  (h transposed,
            stored fp8; k contracted as 3 DoubleRow pairs of 256)
  phase 2:  y[m, h]  = sum_i hT[i, m] * W2[i, h]  (12 DoubleRow pairs of 256);
            z = y + (x + b2);  LayerNorm(z) along h.
"""

import sys

if "/opt/trn_rl_repo" not in sys.path:
    sys.path.insert(0, "/opt/trn_rl_repo")

import numpy as np

def _install_axon_hooks_shim():
    """Provide antenv.axon_hooks (NTFF profiling hook) when the image's
    antenv lacks it — a thin ctypes wrapper over libaxon_pjrt.so, matching
    trn_agent_boot.trn_boot._ntff_profile_via_ctypes.  Only exercised when
    profiling is requested (BASS_TRACE); harmless otherwise."""
    import contextlib
    import ctypes
    import types

    try:
        import antenv.axon_hooks  # noqa: F401
        return
    except ImportError:
        pass
    try:
        import antenv
    except ImportError:
        return

    mod = types.ModuleType("antenv.axon_hooks")
    _state = {"hook": None, "init": False}

    def set_axon_ntff_profile_hook(h):
        _state["hook"] = h
        _state["init"] = True

    def get_axon_ntff_profile_hook():
        if _state["init"]:
            return _state["hook"]
        _state["init"] = True
        try:
            lib = ctypes.CDLL("/opt/axon/libaxon_pjrt.so")
        except OSError:
            return None
        if not hasattr(lib, "axon_start_nrt_profile"):
            return None
        lib.axon_start_nrt_profile.argtypes = [
            ctypes.POINTER(ctypes.c_int64), ctypes.c_size_t]
        lib.axon_start_nrt_profile.restype = ctypes.c_int64
        lib.axon_stop_nrt_profile.argtypes = [ctypes.c_char_p]
        lib.axon_stop_nrt_profile.restype = ctypes.c_int64

        @contextlib.contextmanager
        def _hook(output_dir, device_ids):
            import jax
            jax.devices()
            if device_ids:
                ids = (ctypes.c_int64 * len(device_ids))(*device_ids)
                rc = lib.axon_start_nrt_profile(ids, len(device_ids))
            else:
                rc = lib.axon_start_nrt_profile(None, 0)
            if rc != 0:
                raise RuntimeError(f"axon_start_nrt_profile rc={rc}")
            try:
                yield
            finally:
                n = lib.axon_stop_nrt_profile(str(output_dir).encode())
                print(f"profile: {n} file(s) written to {output_dir}")

        _state["hook"] = _hook
        return _hook

    mod.set_axon_ntff_profile_hook = set_axon_ntff_profile_hook
    mod.get_axon_ntff_profile_hook = get_axon_ntff_profile_hook
    sys.modules["antenv.axon_hooks"] = mod
    antenv.axon_hooks = mod


_install_axon_hooks_shim()

E = 8
N = 8192
H = 768
I = 3072
C = N // E        # 1024 tokens per expert/core
KP = H // 256     # 3   GEMM1 k-pairs (DoubleRow contracts 256 at a time)
IT = I // 128     # 24  i-tiles
IP = I // 256     # 12  GEMM2 k-pairs over the intermediate dim
MT = C // 128     # 8   m-tiles (token dim per core)
# W1 DMA chunk sizes (i-tiles): tiny leading chunks so the first matmul's
# weights land ASAP during the HBM-bound startup burst
W1_CHUNKS = (1, 1, 2, 4, 4, 6, 6)
W2C = 6           # W2 DMA chunks (2 i-pairs each)
# PE warm-up matmuls: tried 8/12/20 — any dummy warm-up correlates with the
# HAM sustaining a ~2.0GHz clock instead of 2.4GHz for the whole kernel
# (low-utilization matmuls appear to poison its duty heuristic). Keep 0.
NDUM = 0
EPS = 1e-12

_CACHE = {}


def _build_nc(act_name="Gelu"):
    from contextlib import ExitStack

    import concourse.tile as tile
    from concourse import bacc, mybir

    f32 = mybir.dt.float32
    f8 = mybir.dt.float8e4
    AF = mybir.ActivationFunctionType
    act_fn = getattr(AF, act_name)
    ALU = mybir.AluOpType
    DR = mybir.MatmulPerfMode.DoubleRow

    nc = bacc.Bacc("TRN2", target_bir_lowering=False, debug=False, num_devices=8)

    # fp8 matmul operands; residual/bias/output stay fp32
    x8 = nc.dram_tensor("x8", [128, KP, 2, C], f8, kind="ExternalInput").ap()
    xn = nc.dram_tensor("xn", [128, MT, H], f32, kind="ExternalInput").ap()
    w1 = nc.dram_tensor("w1", [128, IT, KP, 2, 128], f8,
                        kind="ExternalInput").ap()
    w2 = nc.dram_tensor("w2", [128, W2C, 2, 2, H], f8, kind="ExternalInput").ap()
    b1t = nc.dram_tensor("b1t", [128, IT], f32, kind="ExternalInput").ap()
    out = nc.dram_tensor("out", [128, MT, H], f32, kind="ExternalOutput").ap()

    w1_starts = [sum(W1_CHUNKS[:j]) for j in range(len(W1_CHUNKS))]

    with ExitStack() as ctx:
        tc = ctx.enter_context(tile.TileContext(nc))
        persist = ctx.enter_context(tc.tile_pool(name="persist", bufs=1))
        psum = ctx.enter_context(tc.tile_pool(name="psum", bufs=1, space="PSUM"))
        spool = ctx.enter_context(tc.tile_pool(name="small", bufs=4))

        # per-chunk tiles: Tile RAW deps are tile-granular, so consumers start
        # as soon as their own chunk lands instead of waiting for one big DMA
        hTp = [persist.tile([128, 2, C], f8, name=f"hT{j}", tag=f"hT{j}")
               for j in range(IP)]
        w1c = [persist.tile([128, sz, KP, 2, 128], f8, name=f"w1c{j}",
                            tag=f"w1c{j}") for j, sz in enumerate(W1_CHUNKS)]
        w2c = [persist.tile([128, 2, 2, H], f8, name=f"w2c{j}", tag=f"w2c{j}")
               for j in range(W2C)]
        xc = [persist.tile([128, 2, C], f8, name=f"xk{k}", tag=f"xk{k}")
              for k in range(KP)]
        xns = persist.tile([128, MT, H], f32, name="xns")
        b1s = persist.tile([128, IT], f32, name="b1s")
        epsT = persist.tile([128, 1], f32, name="epsT")
        wz = persist.tile([128, 2, 128], f8, name="wz")

        nc.vector.memset(wz, 0.0)
        nc.vector.memset(epsT, EPS)

        # ---- startup DMAs.  The startup burst is chip-HBM-bound (~350 GB/s
        # per core with all 8 cores bursting) and each dma_start costs ~0.7us
        # of issuing-engine time, so the GEMM1-critical tensors (w1 on sync,
        # x on gpsimd) get dedicated queues; everything phase-2-only (w2 on
        # the idle vector queue, xns on gpsimd) is pinned behind the second
        # gelu so it cannot crowd the burst.  Deps are byte-range-precise:
        # each pinned DMA needs its own dummy write into its own tile.
        nc.gpsimd.dma_start(out=xc[0], in_=x8[:, 0])
        nc.sync.dma_start(out=w1c[0], in_=w1[:, 0:1])
        nc.gpsimd.dma_start(out=xc[1], in_=x8[:, 1])
        nc.gpsimd.dma_start(out=xc[2], in_=x8[:, 2])
        nc.gpsimd.dma_start(out=b1s, in_=b1t)
        for j in range(1, len(W1_CHUNKS)):
            s, sz = w1_starts[j], W1_CHUNKS[j]
            nc.sync.dma_start(out=w1c[j], in_=w1[:, s:s + sz])
        # w2 queued on sync BEHIND all of w1: FIFO order keeps the w1 stream
        # fed at full rate; w2 is only needed from phase 2 (~42us)
        for j in range(W2C):
            nc.sync.dma_start(out=w2c[j], in_=w2[:, j])

        # ---- PE warm-up: dummy matmuls on const zeros ramp the tensor
        # engine's clock (HAM pstate) while the startup DMAs are in flight,
        # so real matmuls start near full rate instead of k=4
        for _ in range(NDUM):
            pd = psum.tile([128, C], f32, name="pd", tag="pt", bufs=4)
            nc.tensor.matmul(pd[:, 0:32], lhsT=wz, rhs=wz[:, :, 0:32],
                             start=True, stop=True, perf_mode=DR)

        # ---- phase 1: hT = gelu(W1.T @ x + b1), fp8 out ----
        def chunk_of(it):
            for j in range(len(W1_CHUNKS) - 1, -1, -1):
                if w1_starts[j] <= it:
                    return j
            raise AssertionError

        for it in range(IT):
            j = chunk_of(it)
            w1t = w1c[j][:, it - w1_starts[j]]         # [128, KP, 2, 128]
            ph = psum.tile([128, C], f32, name="ph", tag="pt", bufs=4)
            for kp in range(KP):
                lhsT = w1t[:, kp]                      # [128, 2, 128]
                for half in range(2):
                    nc.tensor.matmul(
                        ph[:, half * 512:(half + 1) * 512],
                        lhsT=lhsT,
                        rhs=xc[kp][:, :, half * 512:(half + 1) * 512],
                        start=(kp == 0),
                        stop=(kp == KP - 1),
                        perf_mode=DR,
                    )
            nc.scalar.activation(hTp[it // 2][:, it % 2, :], ph, act_fn,
                                 bias=b1s[:, it:it + 1])
            if it == 1:
                # release the phase-2-only residual load now that the
                # startup burst is over (deps are byte-range-precise: the
                # dummy write + single whole-tensor DMA pins all of it)
                nc.scalar.activation(xns[:, 0, 0:1], hTp[0][:, 1, 0:1],
                                     AF.Identity)
                nc.gpsimd.dma_start(out=xns, in_=xn)

        # ---- phase 2: y = hT.T @ W2; z = y + xn; LayerNorm ----
        for mt in range(MT):
            py = psum.tile([128, C], f32, name="py", tag="pt", bufs=4)
            for ip in range(IP):
                lhsT = hTp[ip][:, :, mt * 128:(mt + 1) * 128]   # [128, 2, 128]
                w2t = w2c[ip // 2][:, ip % 2]                   # [128, 2, H]
                nc.tensor.matmul(
                    py[:, 0:512], lhsT=lhsT, rhs=w2t[:, :, 0:512],
                    start=(ip == 0), stop=(ip == IP - 1), perf_mode=DR)
                nc.tensor.matmul(
                    py[:, 512:768], lhsT=lhsT, rhs=w2t[:, :, 512:768],
                    start=(ip == 0), stop=(ip == IP - 1), perf_mode=DR)
            # residual add: z = y + (x + b2)
            z = spool.tile([128, H], f32, name="z", tag="z")
            nc.vector.tensor_add(z, py[:, 0:H], xns[:, mt])
            stats = spool.tile([128, 2, 6], f32, name="stats", tag="stats")
            for sg in range(2):
                nc.vector.bn_stats(stats[:, sg], z[:, sg * 384:(sg + 1) * 384])
            mv = spool.tile([128, 2], f32, name="mv", tag="mv")
            nc.vector.bn_aggr(mv, stats)
            rstd = spool.tile([128, 1], f32, name="rstd", tag="rstd")
            nc.scalar.activation(rstd, mv[:, 1:2], AF.Sqrt, bias=epsT)
            nc.vector.reciprocal(out=rstd, in_=rstd)
            if mt < MT - 1:
                # normalize on the scalar engine ((z-mu)*rstd == z*rstd+nb),
                # freeing the vector engine; store halves overlap compute and
                # alternate gpsimd/sync queues so the final stores parallelize
                nb = spool.tile([128, 1], f32, name="nb", tag="nb")
                nc.vector.tensor_scalar(out=nb, in0=mv[:, 0:1], scalar1=rstd,
                                        scalar2=-1.0, op0=ALU.mult,
                                        op1=ALU.mult)
                for i, h0 in enumerate((0, H // 2)):
                    sl = slice(h0, h0 + H // 2)
                    nc.scalar.activation(z[:, sl], z[:, sl], AF.Identity,
                                         bias=nb, scale=rstd)
                    eng = nc.gpsimd if i == 0 else nc.sync
                    eng.dma_start(out=out[:, mt, sl], in_=z[:, sl])
            else:
                # last tile: the vector engine is free and its tensor_scalar
                # beats the scalar-engine identity on latency
                for i, h0 in enumerate((0, H // 2)):
                    sl = slice(h0, h0 + H // 2)
                    nc.vector.tensor_scalar(
                        out=z[:, sl], in0=z[:, sl], scalar1=mv[:, 0:1],
                        scalar2=rstd, op0=ALU.subtract, op1=ALU.mult)
                    eng = nc.gpsimd if i == 0 else nc.sync
                    eng.dma_start(out=out[:, mt, sl], in_=z[:, sl])

    nc.compile()
    return nc


def _get_nc(act_name="Gelu"):
    key = ("nc", act_name)
    if key not in _CACHE:
        _CACHE[key] = _build_nc(act_name)
    return _CACHE[key]


def _shard_inputs(x, task_ids, W1, b1, W2, b2):
    """Host-side dispatch: stable-sort tokens by expert id, chunk into E
    equal capacity-C blocks (exactly the reference's xs = x[order].reshape),
    and quantize matmul operands to TRN e4m3 fp8."""
    import ml_dtypes

    f8 = ml_dtypes.float8_e4m3
    expert = (task_ids.astype(np.int64) % E).astype(np.int32)
    order = np.argsort(expert, kind="stable")
    xs = x[order]
    in_maps = []
    for e in range(E):
        xe = xs[e * C:(e + 1) * C]                       # [C, H]
        # x8[p, kp, j, m] = q(x)[m, kp*256 + j*128 + p]
        x8 = xe.astype(f8).T.reshape(KP, 2, 128, C).transpose(2, 0, 1, 3)
        xn = (xe + b2[e][None, :]).reshape(MT, 128, H).transpose(1, 0, 2)
        # w1[p, it, kp, j, ci] = q(W1)[kp*256 + j*128 + p, it*128 + ci]
        w1 = (W1[e].astype(f8).reshape(KP, 2, 128, IT, 128)
              .transpose(2, 3, 0, 1, 4))
        # w2[p, c2, s, j, h] = q(W2)[(c2*2+s)*256 + j*128 + p, h]
        w2 = (W2[e].astype(f8).reshape(W2C, 2, 2, 128, H)
              .transpose(3, 0, 1, 2, 4))
        b1t = b1[e].reshape(IT, 128).T
        in_maps.append({
            "x8": np.ascontiguousarray(x8),
            "xn": np.ascontiguousarray(xn, dtype=np.float32),
            "w1": np.ascontiguousarray(w1),
            "w2": np.ascontiguousarray(w2),
            "b1t": np.ascontiguousarray(b1t, dtype=np.float32),
        })
    return in_maps, order


def kernel(x, task_ids, W1, b1, W2, b2, gamma, beta):
    from concourse import bass_utils

    x = np.asarray(x, dtype=np.float32)
    task_ids = np.asarray(task_ids)
    W1 = np.asarray(W1, dtype=np.float32)
    b1 = np.asarray(b1, dtype=np.float32)
    W2 = np.asarray(W2, dtype=np.float32)
    b2 = np.asarray(b2, dtype=np.float32)
    gamma = np.asarray(gamma, dtype=np.float32)
    beta = np.asarray(beta, dtype=np.float32)

    in_maps, order = _shard_inputs(x, task_ids, W1, b1, W2, b2)
    nc = _get_nc()
    res = bass_utils.run_bass_kernel_spmd(nc, in_maps, core_ids=list(range(E)))
    _CACHE["last_results"] = res

    z = np.concatenate(
        [res.results[e]["out"].transpose(1, 0, 2).reshape(C, H) for e in range(E)],
        axis=0)
    # per-expert gamma/beta (identity for this problem's inputs; applied on
    # host only when nontrivial, matching the reference's z*gamma + beta)
    if not (np.all(gamma == 1.0) and np.all(beta == 0.0)):
        blk = np.repeat(np.arange(E), C)  # reference uses capacity blocks
        z = z * gamma[blk] + beta[blk]
    out = np.empty((N, H), dtype=np.float32)
    out[order] = z
    return out



# revision 1
# speedup vs baseline: 1.3808x; 1.3808x over previous
"""MoE FFN (BertGeneration-style) on 8 TRN2 NeuronCores, expert-parallel.

Problem: 8192 tokens, expert = task_id % 8, per-expert FFN
(768 -> 3072 gelu -> 768) + residual + per-expert LayerNorm.

Strategy: routing (dispatch/combine) is a host-side permutation; each of the
8 cores runs one expert's FFN over its 1024-token block.  Matmuls run in
fp8 (e4m3) with perf_mode=DoubleRow: the PE packs two 128-deep k-slices per
pass (256-deep contraction), roughly halving tensor-engine time vs fp32r.
The residual path and LayerNorm stay fp32 (x is added unquantized), so the
fp8 quantization error only enters through y = FFN(x), whose magnitude is
~0.2 of the residual -- overall rel err ~1.4e-2 vs the 2e-2 gate.

On-chip per core:
  phase 1:  hT[i, m] = gelu(sum_k W1[k, i] * xT[k, m] + b1[i])  (h transposed,
            stored fp8; k contracted as 3 DoubleRow pairs of 256)
  phase 2:  y[m, h]  = sum_i hT[i, m] * W2[i, h]  (12 DoubleRow pairs of 256);
            z = y + (x + b2);  LayerNorm(z) along h.
"""

import sys

if "/opt/trn_rl_repo" not in sys.path:
    sys.path.insert(0, "/opt/trn_rl_repo")

import numpy as np

def _install_axon_hooks_shim():
    """Provide antenv.axon_hooks (NTFF profiling hook) when the image's
    antenv lacks it — a thin ctypes wrapper over libaxon_pjrt.so, matching
    trn_agent_boot.trn_boot._ntff_profile_via_ctypes.  Only exercised when
    profiling is requested (BASS_TRACE); harmless otherwise."""
    import contextlib
    import ctypes
    import types

    try:
        import antenv.axon_hooks  # noqa: F401
        return
    except ImportError:
        pass
    try:
        import antenv
    except ImportError:
        return

    mod = types.ModuleType("antenv.axon_hooks")
    _state = {"hook": None, "init": False}

    def set_axon_ntff_profile_hook(h):
        _state["hook"] = h
        _state["init"] = True

    def get_axon_ntff_profile_hook():
        if _state["init"]:
            return _state["hook"]
        _state["init"] = True
        try:
            lib = ctypes.CDLL("/opt/axon/libaxon_pjrt.so")
        except OSError:
            return None
        if not hasattr(lib, "axon_start_nrt_profile"):
            return None
        lib.axon_start_nrt_profile.argtypes = [
            ctypes.POINTER(ctypes.c_int64), ctypes.c_size_t]
        lib.axon_start_nrt_profile.restype = ctypes.c_int64
        lib.axon_stop_nrt_profile.argtypes = [ctypes.c_char_p]
        lib.axon_stop_nrt_profile.restype = ctypes.c_int64

        @contextlib.contextmanager
        def _hook(output_dir, device_ids):
            import jax
            jax.devices()
            if device_ids:
                ids = (ctypes.c_int64 * len(device_ids))(*device_ids)
                rc = lib.axon_start_nrt_profile(ids, len(device_ids))
            else:
                rc = lib.axon_start_nrt_profile(None, 0)
            if rc != 0:
                raise RuntimeError(f"axon_start_nrt_profile rc={rc}")
            try:
                yield
            finally:
                n = lib.axon_stop_nrt_profile(str(output_dir).encode())
                print(f"profile: {n} file(s) written to {output_dir}")

        _state["hook"] = _hook
        return _hook

    mod.set_axon_ntff_profile_hook = set_axon_ntff_profile_hook
    mod.get_axon_ntff_profile_hook = get_axon_ntff_profile_hook
    sys.modules["antenv.axon_hooks"] = mod
    antenv.axon_hooks = mod


_install_axon_hooks_shim()

E = 8
N = 8192
H = 768
I = 3072
C = N // E        # 1024 tokens per expert/core
KP = H // 256     # 3   GEMM1 k-pairs (DoubleRow contracts 256 at a time)
IT = I // 128     # 24  i-tiles
IP = I // 256     # 12  GEMM2 k-pairs over the intermediate dim
MT = C // 128     # 8   m-tiles (token dim per core)
# W1 DMA chunk sizes (i-tiles): tiny leading chunks so the first matmul's
# weights land ASAP during the HBM-bound startup burst
W1_CHUNKS = (1, 1, 2, 4, 4, 6, 6)
W2C = 6           # W2 DMA chunks (2 i-pairs each)
# PE warm-up matmuls: tried 8/12/20 — any dummy warm-up correlates with the
# HAM sustaining a ~2.0GHz clock instead of 2.4GHz for the whole kernel
# (low-utilization matmuls appear to poison its duty heuristic). Keep 0.
NDUM = 0
EPS = 1e-12

_CACHE = {}


def _build_nc(act_name="Gelu"):
    from contextlib import ExitStack

    import concourse.tile as tile
    from concourse import bacc, mybir

    f32 = mybir.dt.float32
    f8 = mybir.dt.float8e4
    AF = mybir.ActivationFunctionType
    act_fn = getattr(AF, act_name)
    ALU = mybir.AluOpType
    DR = mybir.MatmulPerfMode.DoubleRow

    nc = bacc.Bacc("TRN2", target_bir_lowering=False, debug=False, num_devices=8)

    # fp8 matmul operands; residual/bias/output stay fp32
    x8 = nc.dram_tensor("x8", [128, KP, 2, C], f8, kind="ExternalInput").ap()
    xn = nc.dram_tensor("xn", [128, MT, H], f32, kind="ExternalInput").ap()
    w1 = nc.dram_tensor("w1", [128, IT, KP, 2, 128], f8,
                        kind="ExternalInput").ap()
    w2 = nc.dram_tensor("w2", [128, W2C, 2, 2, H], f8, kind="ExternalInput").ap()
    b1t = nc.dram_tensor("b1t", [128, IT], f32, kind="ExternalInput").ap()
    out = nc.dram_tensor("out", [128, MT, H], f32, kind="ExternalOutput").ap()

    w1_starts = [sum(W1_CHUNKS[:j]) for j in range(len(W1_CHUNKS))]

    with ExitStack() as ctx:
        tc = ctx.enter_context(tile.TileContext(nc))
        persist = ctx.enter_context(tc.tile_pool(name="persist", bufs=1))
        psum = ctx.enter_context(tc.tile_pool(name="psum", bufs=1, space="PSUM"))
        spool = ctx.enter_context(tc.tile_pool(name="small", bufs=4))

        # per-chunk tiles: Tile RAW deps are tile-granular, so consumers start
        # as soon as their own chunk lands instead of waiting for one big DMA
        hTp = [persist.tile([128, 2, C], f8, name=f"hT{j}", tag=f"hT{j}")
               for j in range(IP)]
        w1c = [persist.tile([128, sz, KP, 2, 128], f8, name=f"w1c{j}",
                            tag=f"w1c{j}") for j, sz in enumerate(W1_CHUNKS)]
        w2c = [persist.tile([128, 2, 2, H], f8, name=f"w2c{j}", tag=f"w2c{j}")
               for j in range(W2C)]
        xc = [persist.tile([128, 2, C], f8, name=f"xk{k}", tag=f"xk{k}")
              for k in range(KP)]
        xns = persist.tile([128, MT, H], f32, name="xns")
        b1s = persist.tile([128, IT], f32, name="b1s")
        epsT = persist.tile([128, 1], f32, name="epsT")
        wz = persist.tile([128, 2, 128], f8, name="wz")

        nc.vector.memset(wz, 0.0)
        nc.vector.memset(epsT, EPS)

        # ---- startup DMAs.  The startup burst is chip-HBM-bound (~350 GB/s
        # per core with all 8 cores bursting) and each dma_start costs ~0.7us
        # of issuing-engine time, so the GEMM1-critical tensors (w1 on sync,
        # x on gpsimd) get dedicated queues; everything phase-2-only (w2 on
        # the idle vector queue, xns on gpsimd) is pinned behind the second
        # gelu so it cannot crowd the burst.  Deps are byte-range-precise:
        # each pinned DMA needs its own dummy write into its own tile.
        nc.gpsimd.dma_start(out=xc[0], in_=x8[:, 0])
        nc.sync.dma_start(out=w1c[0], in_=w1[:, 0:1])
        nc.gpsimd.dma_start(out=xc[1], in_=x8[:, 1])
        nc.gpsimd.dma_start(out=xc[2], in_=x8[:, 2])
        nc.gpsimd.dma_start(out=b1s, in_=b1t)
        for j in range(1, len(W1_CHUNKS)):
            s, sz = w1_starts[j], W1_CHUNKS[j]
            nc.sync.dma_start(out=w1c[j], in_=w1[:, s:s + sz])
        # w2 queued on sync BEHIND all of w1: FIFO order keeps the w1 stream
        # fed at full rate; w2 is only needed from phase 2 (~42us)
        for j in range(W2C):
            nc.sync.dma_start(out=w2c[j], in_=w2[:, j])

        # ---- PE warm-up: dummy matmuls on const zeros ramp the tensor
        # engine's clock (HAM pstate) while the startup DMAs are in flight,
        # so real matmuls start near full rate instead of k=4
        for _ in range(NDUM):
            pd = psum.tile([128, C], f32, name="pd", tag="pt", bufs=4)
            nc.tensor.matmul(pd[:, 0:32], lhsT=wz, rhs=wz[:, :, 0:32],
                             start=True, stop=True, perf_mode=DR)

        # ---- phase 1: hT = gelu(W1.T @ x + b1), fp8 out ----
        def chunk_of(it):
            for j in range(len(W1_CHUNKS) - 1, -1, -1):
                if w1_starts[j] <= it:
                    return j
            raise AssertionError

        for it in range(IT):
            j = chunk_of(it)
            w1t = w1c[j][:, it - w1_starts[j]]         # [128, KP, 2, 128]
            ph = psum.tile([128, C], f32, name="ph", tag="pt", bufs=4)
            for kp in range(KP):
                lhsT = w1t[:, kp]                      # [128, 2, 128]
                for half in range(2):
                    nc.tensor.matmul(
                        ph[:, half * 512:(half + 1) * 512],
                        lhsT=lhsT,
                        rhs=xc[kp][:, :, half * 512:(half + 1) * 512],
                        start=(kp == 0),
                        stop=(kp == KP - 1),
                        perf_mode=DR,
                    )
            nc.scalar.activation(hTp[it // 2][:, it % 2, :], ph, act_fn,
                                 bias=b1s[:, it:it + 1])
            if it == 1:
                # release the phase-2-only residual load now that the
                # startup burst is over (deps are byte-range-precise: the
                # dummy write + single whole-tensor DMA pins all of it)
                nc.scalar.activation(xns[:, 0, 0:1], hTp[0][:, 1, 0:1],
                                     AF.Identity)
                nc.gpsimd.dma_start(out=xns, in_=xn)

        # ---- phase 2: y = hT.T @ W2; z = y + xn; LayerNorm ----
        for mt in range(MT):
            py = psum.tile([128, C], f32, name="py", tag="pt", bufs=4)
            for ip in range(IP):
                lhsT = hTp[ip][:, :, mt * 128:(mt + 1) * 128]   # [128, 2, 128]
                w2t = w2c[ip // 2][:, ip % 2]                   # [128, 2, H]
                nc.tensor.matmul(
                    py[:, 0:512], lhsT=lhsT, rhs=w2t[:, :, 0:512],
                    start=(ip == 0), stop=(ip == IP - 1), perf_mode=DR)
                nc.tensor.matmul(
                    py[:, 512:768], lhsT=lhsT, rhs=w2t[:, :, 512:768],
                    start=(ip == 0), stop=(ip == IP - 1), perf_mode=DR)
            # residual add: z = y + (x + b2)
            z = spool.tile([128, H], f32, name="z", tag="z")
            nc.vector.tensor_add(z, py[:, 0:H], xns[:, mt])
            stats = spool.tile([128, 2, 6], f32, name="stats", tag="stats")
            for sg in range(2):
                nc.vector.bn_stats(stats[:, sg], z[:, sg * 384:(sg + 1) * 384])
            mv = spool.tile([128, 2], f32, name="mv", tag="mv")
            nc.vector.bn_aggr(mv, stats)
            rstd = spool.tile([128, 1], f32, name="rstd", tag="rstd")
            nc.scalar.activation(rstd, mv[:, 1:2], AF.Sqrt, bias=epsT)
            nc.vector.reciprocal(out=rstd, in_=rstd)
            if mt < MT - 1:
                # normalize on the scalar engine ((z-mu)*rstd == z*rstd+nb),
                # freeing the vector engine; store halves overlap compute and
                # alternate gpsimd/sync queues so the final stores parallelize
                nb = spool.tile([128, 1], f32, name="nb", tag="nb")
                nc.vector.tensor_scalar(out=nb, in0=mv[:, 0:1], scalar1=rstd,
                                        scalar2=-1.0, op0=ALU.mult,
                                        op1=ALU.mult)
                for i, h0 in enumerate((0, H // 2)):
                    sl = slice(h0, h0 + H // 2)
                    nc.scalar.activation(z[:, sl], z[:, sl], AF.Identity,
                                         bias=nb, scale=rstd)
                    eng = nc.gpsimd if i == 0 else nc.sync
                    eng.dma_start(out=out[:, mt, sl], in_=z[:, sl])
            else:
                # last tile: the vector engine is free and its tensor_scalar
                # beats the scalar-engine identity on latency
                for i, h0 in enumerate((0, H // 2)):
                    sl = slice(h0, h0 + H // 2)
                    nc.vector.tensor_scalar(
                        out=z[:, sl], in0=z[:, sl], scalar1=mv[:, 0:1],
                        scalar2=rstd, op0=ALU.subtract, op1=ALU.mult)
                    eng = nc.gpsimd if i == 0 else nc.sync
                    eng.dma_start(out=out[:, mt, sl], in_=z[:, sl])

    nc.compile()
    return nc


def _get_nc(act_name="Gelu"):
    key = ("nc", act_name)
    if key not in _CACHE:
        _CACHE[key] = _build_nc(act_name)
    return _CACHE[key]


def _shard_inputs(x, task_ids, W1, b1, W2, b2):
    """Host-side dispatch: stable-sort tokens by expert id, chunk into E
    equal capacity-C blocks (exactly the reference's xs = x[order].reshape),
    and quantize matmul operands to TRN e4m3 fp8."""
    import ml_dtypes

    f8 = ml_dtypes.float8_e4m3
    expert = (task_ids.astype(np.int64) % E).astype(np.int32)
    order = np.argsort(expert, kind="stable")
    xs = x[order]
    in_maps = []
    for e in range(E):
        xe = xs[e * C:(e + 1) * C]                       # [C, H]
        # x8[p, kp, j, m] = q(x)[m, kp*256 + j*128 + p]
        x8 = xe.astype(f8).T.reshape(KP, 2, 128, C).transpose(2, 0, 1, 3)
        xn = (xe + b2[e][None, :]).reshape(MT, 128, H).transpose(1, 0, 2)
        # w1[p, it, kp, j, ci] = q(W1)[kp*256 + j*128 + p, it*128 + ci]
        w1 = (W1[e].astype(f8).reshape(KP, 2, 128, IT, 128)
              .transpose(2, 3, 0, 1, 4))
        # w2[p, c2, s, j, h] = q(W2)[(c2*2+s)*256 + j*128 + p, h]
        w2 = (W2[e].astype(f8).reshape(W2C, 2, 2, 128, H)
              .transpose(3, 0, 1, 2, 4))
        b1t = b1[e].reshape(IT, 128).T
        in_maps.append({
            "x8": np.ascontiguousarray(x8),
            "xn": np.ascontiguousarray(xn, dtype=np.float32),
            "w1": np.ascontiguousarray(w1),
            "w2": np.ascontiguousarray(w2),
            "b1t": np.ascontiguousarray(b1t, dtype=np.float32),
        })
    return in_maps, order


def kernel(x, task_ids, W1, b1, W2, b2, gamma, beta):
    from concourse import bass_utils

    x = np.asarray(x, dtype=np.float32)
    task_ids = np.asarray(task_ids)
    W1 = np.asarray(W1, dtype=np.float32)
    b1 = np.asarray(b1, dtype=np.float32)
    W2 = np.asarray(W2, dtype=np.float32)
    b2 = np.asarray(b2, dtype=np.float32)
    gamma = np.asarray(gamma, dtype=np.float32)
    beta = np.asarray(beta, dtype=np.float32)

    in_maps, order = _shard_inputs(x, task_ids, W1, b1, W2, b2)
    nc = _get_nc()
    res = bass_utils.run_bass_kernel_spmd(nc, in_maps, core_ids=list(range(E)))
    _CACHE["last_results"] = res

    z = np.concatenate(
        [res.results[e]["out"].transpose(1, 0, 2).reshape(C, H) for e in range(E)],
        axis=0)
    # per-expert gamma/beta (identity for this problem's inputs; applied on
    # host only when nontrivial, matching the reference's z*gamma + beta)
    if not (np.all(gamma == 1.0) and np.all(beta == 0.0)):
        blk = np.repeat(np.arange(E), C)  # reference uses capacity blocks
        z = z * gamma[blk] + beta[blk]
    out = np.empty((N, H), dtype=np.float32)
    out[order] = z
    return out

